# revision 18
# baseline (speedup 1.0000x reference)
"""Trainium2 kernel for nn_AxialAttention_68762426409385.

Strategy (v3): data-parallel over the fused B*T*W row axis, one shard per
NeuronCore, with the ENTIRE computation on device:

  1. Bass/Tile kernel (TensorEngine): the dominant-cost 1x1-conv qkv
     projection, a (1024x512) @ (512 x 8192) bf16 matmul per shard
     (68.7 GFLOP total across cores).
  2. Jitted XLA stages on the same cores, chained device-resident (the
     qkv / sv / sve tensors never leave device HBM): relative-position
     scores, softmax, attention-weighted values; then BatchNorm whose
     per-shard statistics are combined EXACTLY on host (a few KB of
     transfer) and applied on device together with residual + relu.

Host work is only the (B,C,H,W,T) <-> row-major layout permutes, dtype
casts, and the 2KB BN-stat combine. I/O between host and device is bf16
to halve transfer cost; compiled executables are cached persistently so
repeat cold starts are cheap.
"""

import os
import numpy as np
import jax
import jax.numpy as jnp
from jax.sharding import Mesh, PartitionSpec, NamedSharding
from jax.experimental.shard_map import shard_map

# Persistent compilation cache: makes cold start cheap when the NEFFs have
# been compiled before on this machine.
try:
    jax.config.update("jax_compilation_cache_dir",
                      os.path.expanduser("~/.cache/jax_comp_cache"))
    jax.config.update("jax_persistent_cache_min_entry_size_bytes", -1)
    jax.config.update("jax_persistent_cache_min_compile_time_secs", 0.5)
except Exception:
    pass

import concourse.bass as bass
import concourse.bacc as bacc
import concourse.tile as tile
import concourse.mybir as mybir
from concourse.bass2jax import (_bass_exec_p, install_neuronx_cc_hook,
                                partition_id_tensor)

N_HEAD = 8
BN_EPS = 1e-5
B, C, H, W, T = 4, 512, 32, 32, 16
N = B * T * W            # 2048 attention rows
NCORES = 8
NS = N // NCORES         # 256 rows per core
FREE = NS * H            # 8192 columns per core
BLK = 512                # matmul free-dim tile (one fp32 PSUM bank)
NB = FREE // BLK         # 16 blocks

BF16 = mybir.dt.bfloat16
F32 = mybir.dt.float32
NPBF16 = jnp.bfloat16.dtype


def _build_qkv_module():
    nc = bacc.Bacc("TRN2", target_bir_lowering=False)
    xin = nc.dram_tensor("x_sh", [C, FREE], BF16, kind="ExternalInput")
    win = nc.dram_tensor("wT", [C, 2 * C], BF16, kind="ExternalInput")
    qout = nc.dram_tensor("qkv_sh", [2 * C, FREE], BF16, kind="ExternalOutput")

    with tile.TileContext(nc) as tc:
        with tc.tile_pool(name="wp", bufs=1) as wp, \
             tc.tile_pool(name="xp", bufs=8) as xp, \
             tc.tile_pool(name="pp", bufs=4, space="PSUM") as pp, \
             tc.tile_pool(name="op", bufs=4) as op:
            wts = []
            for kc in range(4):
                wt = wp.tile([128, 2 * C], BF16, tag=f"w{kc}")
                nc.sync.dma_start(wt[:], win[kc * 128:(kc + 1) * 128, :])
                wts.append(wt)
            for b in range(NB):
                xts = []
                for kc in range(4):
                    xt = xp.tile([128, BLK], BF16, tag="xt")
                    nc.sync.dma_start(
                        xt[:], xin[kc * 128:(kc + 1) * 128,
                                   b * BLK:(b + 1) * BLK])
                    xts.append(xt)
                for mc in range(8):
                    ps = pp.tile([128, BLK], F32, tag="ps")
                    for kc in range(4):
                        nc.tensor.matmul(
                            ps[:],
                            lhsT=wts[kc][:, mc * 128:(mc + 1) * 128],
                            rhs=xts[kc][:],
                            start=(kc == 0), stop=(kc == 3))
                    ot = op.tile([128, BLK], BF16, tag="ot")
                    nc.any.tensor_copy(ot[:], ps[:])
                    nc.sync.dma_start(
                        qout[mc * 128:(mc + 1) * 128, b * BLK:(b + 1) * BLK],
                        ot[:])
    nc.compile()
    return nc


# ---------------------------------------------------------------------------
# Jitted device pipeline (built once, cached).
# ---------------------------------------------------------------------------

_STATE = {}


def _get_state():
    if _STATE:
        return _STATE
    install_neuronx_cc_hook()
    nc = _build_qkv_module()
    partition_name = (nc.partition_id_tensor.name
                      if nc.partition_id_tensor else None)
    in_names, out_names, out_avals = [], [], []
    for alloc in nc.m.functions[0].allocations:
        if not isinstance(alloc, mybir.MemoryLocationSet):
            continue
        name = alloc.memorylocations[0].name
        if alloc.kind == "ExternalInput":
            if name != partition_name:
                in_names.append(name)
        elif alloc.kind == "ExternalOutput":
            out_names.append(name)
            out_avals.append(jax.core.ShapedArray(
                tuple(alloc.tensor_shape), mybir.dt.np(alloc.dtype)))
    all_in = list(in_names) + list(out_names)
    if partition_name is not None:
        all_in.append(partition_name)

    def bass_body(x_sh, wT, zeros):
        ops = [x_sh, wT, zeros]
        if partition_name is not None:
            ops.append(partition_id_tensor())
        outs = _bass_exec_p.bind(
            *ops, out_avals=tuple(out_avals), in_names=tuple(all_in),
            out_names=tuple(out_names), lowering_input_output_aliases=(),
            sim_require_finite=True, sim_require_nnan=True, nc=nc)
        return outs[0]

    def tail_a_body(qkv_sh, all_emb):
        # qkv_sh [2C, FREE] bf16 (shard-local). Scores, softmax, attention.
        qkv = qkv_sh.astype(jnp.float32).reshape(N_HEAD, 128, NS, H)
        q = qkv[:, :32]
        k = qkv[:, 32:64]
        v = qkv[:, 64:]
        q_emb = all_emb[:32]
        k_emb = all_emb[32:64]
        v_emb = all_emb[64:]
        qk = jnp.einsum('hcni,hcnj->hnij', q, k)
        qr = jnp.einsum('hcni,cij->hnij', q, q_emb)
        kr = jnp.einsum('hcni,cij->hnij', k, k_emb)
        sim = jax.nn.softmax(qk + qr + kr, axis=3)
        sv = jnp.einsum('hnij,hcnj->hcni', sim, v)       # (8,64,NS,H)
        sve = jnp.einsum('hnij,cij->hcni', sim, v_emb)
        # local BN partials (combined exactly on host across cores)
        s1 = sv.sum(axis=(2, 3))
        sq1 = (sv * sv).sum(axis=(2, 3))
        s2 = sve.sum(axis=(2, 3))
        sq2 = (sve * sve).sum(axis=(2, 3))
        stats = jnp.stack([s1, sq1, s2, sq2])            # (4, 8, 64)
        return sv, sve, stats

    def tail_b_body(sv, sve, x_sh, a1, a2, shift):
        xr = x_sh.astype(jnp.float32).reshape(N_HEAD, 64, NS, H)
        out = (sv * a1[:, :, None, None] + sve * a2[:, :, None, None]
               - shift[:, :, None, None] + xr)
        out = jnp.maximum(out, 0.0)
        return out.reshape(C, FREE).astype(jnp.bfloat16)

    devices = jax.devices()[:NCORES]
    mesh = Mesh(np.asarray(devices), ("core",))
    PS = PartitionSpec
    shard = NamedSharding(mesh, PS("core"))
    repl = NamedSharding(mesh, PS())

    bass_fn = jax.jit(
        shard_map(bass_body, mesh=mesh,
                  in_specs=(PS("core"),) * 3, out_specs=PS("core"),
                  check_rep=False),
        donate_argnums=(2,), keep_unused=True)
    tail_a_fn = jax.jit(
        shard_map(tail_a_body, mesh=mesh,
                  in_specs=(PS("core"), PS()),
                  out_specs=(PS("core"), PS("core"), PS("core")),
                  check_rep=False))
    tail_b_fn = jax.jit(
        shard_map(tail_b_body, mesh=mesh,
                  in_specs=(PS("core"), PS("core"), PS("core"), PS(), PS(),
                            PS()),
                  out_specs=PS("core"), check_rep=False),
        donate_argnums=(0,), keep_unused=True)

    _STATE.update(dict(bass_fn=bass_fn, tail_a_fn=tail_a_fn,
                       tail_b_fn=tail_b_fn, mesh=mesh,
                       shard=shard, repl=repl,
                       out_shape=tuple(out_avals[0].shape),
                       out_dtype=out_avals[0].dtype))
    return _STATE


def _prep_host(x, w_qkv, relative, bn_gamma, bn_beta):
    """Host-side prep: permutes + dtype casts + constant tables."""
    # cast first (halves the bytes the permute moves), then permute
    xbf = x.astype(NPBF16)                                 # (B,C,H,W,T)
    # rows n=(b,t,w) shard s covers n in [s*NS,(s+1)*NS); per-core layout
    # (C, (n_local, h)): (B,C,H,W,T) -> (B,T,W,C,H) -> (cores, NS, C, H)
    xp_rows = np.ascontiguousarray(np.transpose(xbf, (0, 4, 3, 1, 2))
                                   ).reshape(NCORES, NS, C, H)
    xs_cat = np.ascontiguousarray(xp_rows.transpose(0, 2, 1, 3)
                                  ).reshape(NCORES * C, FREE)
    wT = np.ascontiguousarray(w_qkv.T).astype(NPBF16)
    wt_cat = np.concatenate([wT] * NCORES, axis=0)

    ar = np.arange(H)
    rel_idx = ar[:, None] - ar[None, :] + H - 1
    all_emb = np.ascontiguousarray(relative[:, rel_idx]).astype(np.float32)

    hh = np.arange(N_HEAD)[:, None]
    ch = np.arange(64)[None, :]
    g1 = bn_gamma[hh * 128 + 2 * ch].astype(np.float32)
    g2 = bn_gamma[hh * 128 + 2 * ch + 1].astype(np.float32)
    b1 = bn_beta[hh * 128 + 2 * ch].astype(np.float32)
    b2 = bn_beta[hh * 128 + 2 * ch + 1].astype(np.float32)
    return xs_cat, wt_cat, all_emb, g1, g2, b1, b2


_DEV_CACHE = {}


def _dev_cached(name, arr, sharding):
    """Device-resident cache for invariant tensors (weights/constants),
    keyed on content so a changed input re-uploads."""
    key = (name, arr.shape, str(arr.dtype), hash(arr.tobytes()))
    hit = _DEV_CACHE.get(name)
    if hit is not None and hit[0] == key:
        return hit[1]
    dev = jax.device_put(arr, sharding)
    _DEV_CACHE[name] = (key, dev)
    return dev


def _host_bn_coeffs(local_stats, g1, g2, b1, b2):
    """Exact global BN coefficients from per-core partial sums."""
    s1, sq1, s2, sq2 = local_stats.sum(axis=0)
    cnt = float(N * H)
    m1 = s1 / cnt
    va1 = np.maximum(sq1 / cnt - m1 * m1, 0.0)
    m2 = s2 / cnt
    va2 = np.maximum(sq2 / cnt - m2 * m2, 0.0)
    a1 = (g1 / np.sqrt(va1 + BN_EPS)).astype(np.float32)
    a2 = (g2 / np.sqrt(va2 + BN_EPS)).astype(np.float32)
    shift = (a1 * m1 + a2 * m2 - b1 - b2).astype(np.float32)
    return a1, a2, shift


def _run_device(xs_cat, wt_cat, all_emb, g1, g2, b1, b2):
    """Full device phase: H2D, bass qkv, XLA attention, BN (stats combined
    exactly on host across the 8 shards), residual+relu, D2H.

    Returns out_cat (NCORES*C, FREE) bf16 numpy.

    Note: a merged single-dispatch tail using an on-device lax.psum for the
    BN statistics was measured SLOWER here (3.47s vs 3.24s): on this
    transfer tunnel the collective plus its verification fetches cost more
    than the one dispatch they save, and the split path keeps the BN
    combine exact and deterministic on host.
    """
    st = _get_state()
    zeros = jnp.zeros((NCORES * st["out_shape"][0],) + st["out_shape"][1:],
                      st["out_dtype"])
    x_dev = jax.device_put(xs_cat, st["shard"])
    wt_dev = _dev_cached("wT", wt_cat, st["shard"])
    emb_dev = _dev_cached("all_emb", all_emb, st["repl"])
    qkv_cat = st["bass_fn"](x_dev, wt_dev, zeros)
    sv, sve, stats_cat = st["tail_a_fn"](qkv_cat, emb_dev)
    local = np.asarray(stats_cat).reshape(NCORES, 4, N_HEAD, 64)
    a1, a2, shift = _host_bn_coeffs(local, g1, g2, b1, b2)
    out_cat = st["tail_b_fn"](sv, sve, x_dev, a1, a2, shift)
    # prefetch all 8 shards concurrently (pipelined D2H beats the strict
    # request-response serialization of a plain np.asarray by ~0.2-0.7s)
    try:
        for s in out_cat.addressable_shards:
            s.data.copy_to_host_async()
        out_np = np.empty(out_cat.shape, out_cat.dtype)
        for s in out_cat.addressable_shards:
            out_np[s.index] = np.asarray(s.data)
        return out_np
    except Exception:
        return np.asarray(out_cat)


def _finish_host(out_cat, x):
    # permute in bf16 (half the bytes), upcast once at the end
    out = out_cat.reshape(NCORES, C, NS, H)
    out = out.transpose(0, 2, 1, 3).reshape(B, T, W, C, H)
    out = np.ascontiguousarray(out.transpose(0, 3, 4, 2, 1))  # (B,C,H,W,T)
    return out.astype(np.float32)


def kernel(x, w_qkv, relative, bn_gamma, bn_beta):
    x = np.asarray(x, dtype=np.float32)
    w_qkv = np.asarray(w_qkv, dtype=np.float32)
    relative = np.asarray(relative, dtype=np.float32)
    bn_gamma = np.asarray(bn_gamma, dtype=np.float32)
    bn_beta = np.asarray(bn_beta, dtype=np.float32)

    prepped = _prep_host(x, w_qkv, relative, bn_gamma, bn_beta)
    out_cat = _run_device(*prepped)
    return _finish_host(out_cat, x)
